# revision 1
# baseline (speedup 1.0000x reference)
"""Trainium2 Bass kernel for nn_Conv_block_57690000720236.

Reference computation (per batch image b):
  - 3x3 SAME conv "high" branch: 64ch -> 64ch
  - low branch: 3x3 conv 64ch -> 16ch, then 1x1 conv 16ch -> 64ch
  - output position (b,y,x) takes the high value if its flat index is in
    mask_idx, the low value if in inv_mask_idx (inv wins on overlap), 0 if
    in neither.

Strategy (8 NeuronCores, data-parallel over batch):
  - Core b computes BOTH branches densely for image b. The low branch is
    folded on the host: W_low = w2 @ w1 (exact up to fp32 rounding), so
    both branches are plain 3x3 convs computed in one set of matmuls with
    M=128 output columns (64 high + 64 low).
  - Layout: channels on SBUF partitions. Partitions 0-63 hold the
    zero-padded image P (130 cols), partitions 64-127 hold P shifted down
    one row (the host stages this duplicated layout in DRAM so one linear
    DMA fills both halves). A K=128 fp32r matmul contracts the (ky, ky+1)
    tap pairs; leftover ky=2 taps run as K=64 matmuls on PE row group 0.
  - Routing is a per-position select between the PSUM halves: the high
    half is evicted straight into the per-tile output buffer, the low
    half is moved to partitions 0-63 with a tiny fp32r identity matmul,
    and one DVE copy_predicated per chunk applies the host-built uint8
    routing mask. Output is written back with one DMA per row-tile.
"""

import numpy as np

import concourse.bacc as bacc
import concourse.mybir as mybir
import concourse.tile as tile
from concourse.bass_utils import run_bass_kernel_spmd

B, CIN, H, W = 8, 64, 128, 128
COUT, KER = 64, 3
NPOS = H * W                 # 16384 positions per core
WP = W + 2                   # padded row length 130
N_TILES = 8                  # image row-tiles held in SBUF
TILE_OUT_ROWS = H // N_TILES     # 16 output rows per tile
TILE_P_ROWS = TILE_OUT_ROWS + 2  # 18 padded rows held per tile
CHUNK_ROWS = 4               # output rows per matmul chunk
CHUNK = CHUNK_ROWS * W       # 512 positions per chunk
CHUNKS_PER_TILE = TILE_OUT_ROWS // CHUNK_ROWS
TILE_POS = TILE_OUT_ROWS * W     # 2048 positions per tile
F32 = mybir.dt.float32
F32R = mybir.dt.float32r
F16 = mybir.dt.float16
U8 = mybir.dt.uint8
WBLK = 6 * 128               # weight blob: 6 matmul blocks
WCOLS = WBLK + 64            # + identity block


def _build_program(need_zero_fix: bool):
    nc = bacc.Bacc("TRN2", target_bir_lowering=False, debug=False, num_devices=B)

    inx_d = nc.dram_tensor(
        "inxs", [N_TILES, 128, TILE_P_ROWS * WP], F16, kind="ExternalInput"
    )
    w_d = nc.dram_tensor("wblob", [128, WCOLS], F16, kind="ExternalInput")
    m_d = nc.dram_tensor("mlow", [COUT, NPOS], U8, kind="ExternalInput")
    if need_zero_fix:
        mz_d = nc.dram_tensor("mzero", [COUT, NPOS], U8, kind="ExternalInput")
    out_d = nc.dram_tensor("out", [COUT, NPOS], F32, kind="ExternalOutput")

    with tile.TileContext(nc) as tc:
        with (
            tc.tile_pool(name="const", bufs=1) as cpool,
            tc.tile_pool(name="img", bufs=1) as ipool,
            tc.tile_pool(name="work", bufs=4) as wkpool,
            tc.tile_pool(name="outp", bufs=2) as opool,
            tc.tile_pool(name="psum", bufs=5, space="PSUM") as pspool,
            tc.tile_pool(name="psum2", bufs=2, space="PSUM") as ps2pool,
        ):
            wt = cpool.tile([128, WCOLS], F16, tag="wblob")
            nc.gpsimd.dma_start(wt[:], w_d[:])
            mt = cpool.tile([COUT, NPOS], U8, tag="mlow")
            nc.gpsimd.dma_start(mt[:], m_d[:])
            if need_zero_fix:
                mzt = cpool.tile([COUT, NPOS], U8, tag="mzero")
                nc.gpsimd.dma_start(mzt[:], mz_d[:])
                zt = cpool.tile([COUT, CHUNK], F32, tag="zeros")
                nc.any.memset(zt[:], 0.0)

            for i in range(N_TILES):
                img = ipool.tile([128, TILE_P_ROWS * WP], F16, tag=f"img{i}")
                # two DMAs (one per partition half) fill the staged layout
                nc.sync.dma_start(img[0:64, :], inx_d[i, 0:64, :])
                nc.sync.dma_start(img[64:128, :], inx_d[i, 64:128, :])
                v = img[:].rearrange("p (r x) -> p r x", x=WP)

                out_sb = opool.tile([COUT, TILE_POS], F32, tag="osb")

                for j in range(CHUNKS_PER_TILE):
                    l0 = j * CHUNK_ROWS       # local output row within tile
                    so = j * CHUNK            # position offset within tile
                    s = i * TILE_POS + so     # global flat position offset

                    pt = pspool.tile([128, CHUNK], F32, tag="acc")
                    pv = pt[:].rearrange("p (r x) -> p r x", x=W)

                    # ky in {0,1} tap pairs: K=128 matmuls, one per kx
                    for c in range(3):
                        nc.tensor.matmul(
                            pv,
                            wt[:, c * 128:(c + 1) * 128],
                            v[:, l0:l0 + CHUNK_ROWS, c:c + W],
                            start=(c == 0),
                            stop=False,
                        )
                    # ky=2 taps: K=64 matmuls on PE row group 0 (A half)
                    for c in range(3):
                        nc.tensor.matmul(
                            pv,
                            wt[0:64, (3 + c) * 128:(4 + c) * 128],
                            v[0:64, l0 + 2:l0 + 2 + CHUNK_ROWS, c:c + W],
                            start=False,
                            stop=(c == 2),
                        )

                    # high half -> output buffer (base value)
                    nc.any.tensor_copy(out_sb[:, so:so + CHUNK], pt[0:64, :])
                    # low half -> partitions 0-63 via fp32r identity matmul
                    lowtmp = wkpool.tile([128, CHUNK], F16, tag="lowtmp")
                    nc.vector.tensor_copy(lowtmp[64:128, :], pt[64:128, :])
                    p2 = ps2pool.tile([COUT, CHUNK], F32, tag="acc2")
                    nc.tensor.matmul(
                        p2[:],
                        wt[64:128, WBLK:WBLK + 64],
                        lowtmp[64:128, :],
                        start=True,
                        stop=True,
                    )
                    # apply routing mask
                    nc.vector.copy_predicated(
                        out_sb[:, so:so + CHUNK], mt[:, s:s + CHUNK], p2[:]
                    )
                    if need_zero_fix:
                        nc.vector.copy_predicated(
                            out_sb[:, so:so + CHUNK], mzt[:, s:s + CHUNK], zt[:]
                        )

                nc.scalar.dma_start(
                    out_d[:, i * TILE_POS:(i + 1) * TILE_POS], out_sb[:]
                )

    nc.compile()
    return nc


def _prepare_host(inx, mask_idx, inv_mask_idx, high_w, low1_w, low2_w):
    inx = np.asarray(inx, dtype=np.float32)
    mask_idx = np.asarray(mask_idx).astype(np.int64)
    inv_mask_idx = np.asarray(inv_mask_idx).astype(np.int64)
    high_w = np.asarray(high_w, dtype=np.float32)
    low1_w = np.asarray(low1_w, dtype=np.float32)
    low2_w = np.asarray(low2_w, dtype=np.float32)

    # zero-padded images P [B, 64, 130, 130]
    inxp = np.zeros((B, CIN, H + 2, WP), np.float32)
    inxp[:, :, 1:-1, 1:-1] = inx
    # staged layout: tile i partitions 0-63 = P rows 16i..16i+17,
    # partitions 64-127 = the same shifted down one row
    stage = np.zeros((B, N_TILES, 128, TILE_P_ROWS, WP), np.float16)
    for i in range(N_TILES):
        tp = i * TILE_OUT_ROWS
        stage[:, i, 0:64] = inxp[:, :, tp:tp + TILE_P_ROWS]
        nb = min(TILE_P_ROWS, (H + 2) - (tp + 1))
        stage[:, i, 64:128, :nb] = inxp[:, :, tp + 1:tp + 1 + nb]
    stage = stage.reshape(B, N_TILES, 128, TILE_P_ROWS * WP)

    # fold the low branch: W_low[o, c, ky, kx] = sum_m w2[o, m] w1[m, c, ky, kx]
    w2 = low2_w.reshape(COUT, -1).astype(np.float64)
    wl = np.einsum("om,mckl->ockl", w2, low1_w.astype(np.float64)).astype(np.float32)
    wh = high_w

    # weight blob [128, 6*128+64]:
    #   block c in {0,1,2}: K=128 pair taps (rows 0-63 ky=0, rows 64-127 ky=1)
    #   block 3+c: rows 0-63 = (ky=2, kx=c) for PE row group 0
    #   cols 768..831 rows 64-127: identity (cross-partition low move)
    # lhsT[k, m]: k = input channel, m = output column (0-63 high, 64-127 low)
    blob = np.zeros((128, WCOLS), np.float16)
    for c in range(3):
        blk = blob[:, c * 128:(c + 1) * 128]
        blk[0:64, 0:64] = wh[:, :, 0, c].T
        blk[0:64, 64:128] = wl[:, :, 0, c].T
        blk[64:128, 0:64] = wh[:, :, 1, c].T
        blk[64:128, 64:128] = wl[:, :, 1, c].T
        sblk = blob[:, (3 + c) * 128:(4 + c) * 128]
        sblk[0:64, 0:64] = wh[:, :, 2, c].T
        sblk[0:64, 64:128] = wl[:, :, 2, c].T
    blob[64:128, WBLK:WBLK + 64] = np.eye(64, dtype=np.float16)

    ntotal = B * NPOS
    in_mask = np.zeros(ntotal, dtype=bool)
    in_inv = np.zeros(ntotal, dtype=bool)
    in_mask[mask_idx] = True
    in_inv[inv_mask_idx] = True
    neither = ~(in_mask | in_inv)
    need_zero_fix = bool(neither.any())

    in_maps = []
    for b in range(B):
        sl = slice(b * NPOS, (b + 1) * NPOS)
        mlow = np.ascontiguousarray(
            np.broadcast_to(in_inv[sl].astype(np.uint8)[None, :], (COUT, NPOS))
        )
        m = {"inxs": stage[b], "wblob": blob, "mlow": mlow}
        if need_zero_fix:
            m["mzero"] = np.ascontiguousarray(
                np.broadcast_to(neither[sl].astype(np.uint8)[None, :], (COUT, NPOS))
            )
        in_maps.append(m)
    return in_maps, need_zero_fix


def _run(inputs: dict, trace: bool = False):
    in_maps, need_zero_fix = _prepare_host(**inputs)
    nc = _build_program(need_zero_fix)
    res = run_bass_kernel_spmd(nc, in_maps, list(range(B)), trace=trace)
    out = np.stack(
        [res.results[b]["out"].reshape(COUT, H, W) for b in range(B)]
    ).astype(np.float32)
    return out, res


def kernel(**inputs) -> np.ndarray:
    out, _ = _run(inputs, trace=False)
    return out

